# revision 1
# baseline (speedup 1.0000x reference)
"""Trainium2 Bass kernel for nn_Cross_head (sparse_attention patch-correction).

Math (non-overlapping unfold/fold are inverse permutations, so
corr = y + fold(attentions/nz)):
    y   = W @ x + b                       (1x1x1 conv over channels)
    out = leaky_relu(y * (y + foldA + 1), 0.2)
    foldA = fold(attentions / (count_nonzero(attentions, -1) + 1e-5))

Sharding: spatial, across the 576 patch-columns (72 per core); no
cross-core communication.  Every DMA transfer is contiguous on the DRAM
side.  Per subtile (9 d-rows x 36 patch columns) the matmul streams x's
columns one d-row at a time in (p2, iW) order (innermost count 36, even,
which float32r requires), so PSUM holds y in (p1, p2, iW) order; the
patch<->psum<->voxel permutations ride on strided access patterns of ops
we need anyway (the att*1/nz multiply and the final leaky-relu).

Engine assignment per subtile:
  sync   : att load, out store (HWDGE)
  gpsimd : x load with fp32->fp32r cast (SWDGE), W load+cast
  tensor : 9 matmuls (f32r, full-rate) into 3 psum groups
  scalar : Sign(att) for the nonzero count; final Prelu(alpha=.2)
  vector : segmented reduce -> 1/nz, A = att*r, t = (A+(b+1))+psum,
           pre = (psum+b)*t
"""

import os
import sys

import numpy as np

sys.path.insert(0, "/opt/trn_rl_repo")

# ---- geometry (hardcoded for this problem) ----
C = 128          # channels (in == out)
D = 36           # depth
HWFULL = 5184    # H*W = 72*72
PS = 9           # patch size
NDP = 4          # D // PS
NWP = 576        # HWFULL // PS  (patch columns)
NCORES = 8
IWG = NWP // NCORES   # 72 patch columns per core
HWL = IWG * PS        # 648 voxel columns per core
NSUB = 2              # split each iD block into halves along iW
IWT = IWG // NSUB     # 36 patch columns per subtile
FT = IWT * 81         # 2916 free elements per subtile
HWT = IWT * PS        # 324 voxel columns per subtile
MMN = 486             # matmul free dim: arbitrary contiguous voxel slice
NMM = PS * HWT // MMN # 6 matmuls per subtile
NGRP = 2              # psum groups per subtile
MMG = NMM // NGRP     # 3 matmuls per psum group
BANK = 512            # fp32 elements per PSUM bank

_NC_CACHE = {}
LAST_RESULT = None


def _build_nc(mm_dtype="float32r", amul_engine="gpsimd"):
    from contextlib import ExitStack

    import concourse.bacc as bacc
    import concourse.tile as tile
    from concourse import mybir

    f32 = mybir.dt.float32
    bf16 = mybir.dt.bfloat16
    AL = mybir.AluOpType
    AF = mybir.ActivationFunctionType

    nc = bacc.Bacc(
        "TRN2",
        target_bir_lowering=False,
        debug=False,
        enable_asserts=False,
        num_devices=NCORES,
    )
    x_d = nc.dram_tensor("x", [C, D, HWL], f32, kind="ExternalInput").ap()
    a_d = nc.dram_tensor("att", [C, NDP, IWG * 81], f32, kind="ExternalInput").ap()
    wt_d = nc.dram_tensor("wt", [C, C], f32, kind="ExternalInput").ap()
    b_d = nc.dram_tensor("bias", [C, 2], f32, kind="ExternalInput").ap()
    o_d = nc.dram_tensor("out", [C, D, HWL], f32, kind="ExternalOutput").ap()

    mmdt = f32 if mm_dtype == "float32" else getattr(mybir.dt, mm_dtype)
    mm_cast = mm_dtype != "float32"

    with tile.TileContext(nc) as tc, ExitStack() as ctx:
        const = ctx.enter_context(tc.tile_pool(name="const", bufs=1))
        wt_sb = const.tile([C, C], mmdt)
        if mm_cast:
            nc.gpsimd.dma_start(wt_sb[:], wt_d[:])  # casts during DMA
        else:
            nc.sync.dma_start(wt_sb[:], wt_d[:])
        b_sb = const.tile([C, 2], f32)
        nc.sync.dma_start(b_sb[:], b_d[:])
        b_ap = b_sb[:, 0:1]
        bp1_ap = b_sb[:, 1:2]
        alpha_sb = const.tile([C, 1], f32)
        nc.vector.memset(alpha_sb[:], 0.2)

        xp = ctx.enter_context(tc.tile_pool(name="xp", bufs=3))
        atp = ctx.enter_context(tc.tile_pool(name="atp", bufs=3))
        sgp = ctx.enter_context(tc.tile_pool(name="sgp", bufs=2))
        nzp = ctx.enter_context(tc.tile_pool(name="nzp", bufs=2))
        Apl = ctx.enter_context(tc.tile_pool(name="Apl", bufs=2))
        tpl = ctx.enter_context(tc.tile_pool(name="tpl", bufs=2))
        prp = ctx.enter_context(tc.tile_pool(name="prp", bufs=2))
        ovp = ctx.enter_context(tc.tile_pool(name="ovp", bufs=3))
        psp = ctx.enter_context(tc.tile_pool(name="psp", bufs=2, space="PSUM"))

        NT = NDP * NSUB

        def issue_loads(sub):
            iD, h = divmod(sub, NSUB)
            xt = xp.tile([C, PS * HWT], mmdt, name=f"xt{sub}", tag="xt")
            xsrc = x_d[:, iD * PS : (iD + 1) * PS, h * HWT : (h + 1) * HWT]
            if mm_cast:
                nc.gpsimd.dma_start(xt[:], xsrc)  # casts during DMA
            else:
                nc.sync.dma_start(xt[:], xsrc)
            at = atp.tile([C, FT], f32, name=f"at{sub}", tag="at")
            nc.sync.dma_start(at[:], a_d[:, iD, h * FT : (h + 1) * FT])
            return xt, at

        loaded = {0: issue_loads(0), 1: issue_loads(1)}

        for sub in range(NT):
            iD, h = divmod(sub, NSUB)
            if amul_engine == "split":
                amul = nc.gpsimd if sub % 2 == 0 else nc.vector
            elif amul_engine == "gpsimd":
                amul = nc.gpsimd
            else:
                amul = nc.vector
            xt, at = loaded.pop(sub)
            if sub + 2 < NT:
                loaded[sub + 2] = issue_loads(sub + 2)

            if True:
                # ---- nz = count_nonzero per patch:  sum |sign(att)| ----
                st = sgp.tile([C, FT], bf16)
                nc.scalar.activation(st[:], at[:], AF.Sign)
                nzv = nzp.tile([C, IWT], f32)
                nc.vector.tensor_reduce(
                    nzv[:],
                    st[:].rearrange("c (w k) -> c w k", k=81),
                    mybir.AxisListType.X,
                    AL.add,
                    apply_absolute_value=True,
                )
                nzr = nzp.tile([C, IWT], f32)
                nc.vector.tensor_scalar_add(nzv[:], nzv[:], 1e-5)
                nc.vector.reciprocal_approx_fast(nzr[:], nzv[:])

                # ---- A = att * (1/nz), written in VOXEL order (p1, iW, p2) -
                # at free layout is (iW:36, p1:9, p2:9); read it strided in
                # (p1, iW, p2) order (innermost contiguous runs of 9), write
                # contiguous.  Everything downstream is then voxel-ordered.
                At = Apl.tile([C, FT], f32)
                a3 = (
                    at[:]
                    .rearrange("c (iw p q) -> c iw p q", p=PS, q=PS)
                    .transpose([0, 2, 1, 3])
                )  # dims (p1, iW, p2) steps (9, 81, 1)
                nzr3 = (
                    nzr[:]
                    .unsqueeze(1)
                    .unsqueeze(3)
                    .broadcast_to((C, PS, IWT, PS))
                )
                A3 = At[:].rearrange("c (p iw q) -> c p iw q", p=PS, q=PS)
                amul.tensor_tensor(A3, a3, nzr3, AL.mult)

                # ---- GEMM: psum = W @ x, plain contiguous voxel slices ----
                pst = []
                for g in range(NGRP):
                    ps_t = psp.tile([C, MMG * BANK], f32)  # 3 banks
                    pst.append(ps_t)
                    for m in range(MMG):
                        ch = g * MMG + m
                        nc.tensor.matmul(
                            ps_t[:, m * BANK : m * BANK + MMN],
                            wt_sb[:],
                            xt[:, ch * MMN : (ch + 1) * MMN],
                            start=True,
                            stop=True,
                        )

                # ---- t = (A + (b+1)) + psum ; pre = (psum + b) * t ----
                tt = tpl.tile([C, FT], f32)
                pre = prp.tile([C, FT], f32)
                for g in range(NGRP):
                    ps_ap = (
                        pst[g][:]
                        .rearrange("c (m n) -> c m n", n=BANK)[:, :, 0:MMN]
                    )  # [C, 3, 324]
                    sl = slice(g * MMG * MMN, (g + 1) * MMG * MMN)
                    A2 = At[:, sl].rearrange("c (m n) -> c m n", n=MMN)
                    t2 = tt[:, sl].rearrange("c (m n) -> c m n", n=MMN)
                    p2_ = pre[:, sl].rearrange("c (m n) -> c m n", n=MMN)
                    nc.vector.scalar_tensor_tensor(
                        t2, A2, bp1_ap, ps_ap, AL.add, AL.add
                    )
                    nc.vector.scalar_tensor_tensor(
                        p2_, ps_ap, b_ap, t2, AL.add, AL.mult
                    )

                # ---- out = lrelu(pre); already voxel order, contiguous ----
                ov = ovp.tile([C, PS * HWT], f32)
                nc.scalar.activation(
                    ov[:], pre[:], AF.Prelu, alpha=alpha_sb[:, 0:1]
                )

                # ---- contiguous store ----
                nc.sync.dma_start(
                    o_d[:, iD * PS : (iD + 1) * PS, h * HWT : (h + 1) * HWT], ov[:]
                )

    nc.compile()
    return nc


def _get_nc(**kw):
    key = tuple(sorted(kw.items()))
    if key not in _NC_CACHE:
        _NC_CACHE[key] = _build_nc(**kw)
    return _NC_CACHE[key]


def kernel(x, attentions, W, b, **build_kw):
    global LAST_RESULT
    from concourse.bass_utils import run_bass_kernel_spmd

    x = np.asarray(x, dtype=np.float32)
    attentions = np.asarray(attentions, dtype=np.float32)
    W = np.asarray(W, dtype=np.float32)
    b = np.asarray(b, dtype=np.float32)

    nc = _get_nc(**build_kw)

    xs = x.reshape(C, D, NCORES, HWL)
    as4 = attentions.reshape(C, NDP, NCORES, IWG, 81)
    wt = np.ascontiguousarray(W.T)
    bcol = np.ascontiguousarray(np.stack([b, b + 1.0], axis=1))

    in_maps = []
    for s in range(NCORES):
        in_maps.append(
            {
                "x": np.ascontiguousarray(xs[:, :, s, :]),
                "att": np.ascontiguousarray(as4[:, :, s, :, :]).reshape(
                    C, NDP, IWG * 81
                ),
                "wt": wt,
                "bias": bcol,
            }
        )

    res = run_bass_kernel_spmd(
        nc,
        in_maps,
        core_ids=list(range(NCORES)),
        trace=bool(os.environ.get("BASS_TRACE")),
    )
    LAST_RESULT = res

    out = np.empty((C, D, NCORES, HWL), dtype=np.float32)
    for s in range(NCORES):
        out[:, :, s, :] = res.results[s]["out"]
    return out.reshape(1, C, D, HWFULL)



# revision 10
# speedup vs baseline: 1.0666x; 1.0666x over previous
"""Trainium2 Bass kernel for nn_Cross_head (sparse_attention patch-correction).

Math (non-overlapping unfold/fold are inverse permutations, so
corr = y + fold(attentions/nz)):
    y   = W @ x + b                       (1x1x1 conv over channels)
    out = leaky_relu(y * (y + foldA + 1), 0.2)
    foldA = fold(attentions / (count_nonzero(attentions, -1) + 1e-5))

Sharding: spatial, across the 576 patch-columns (72 per core); no
cross-core communication.

The kernel is DMA-bound, so inputs are staged in bf16 (host casts) and
the output is stored in bf16 and upcast on the host; tolerance is 2e-2
and bf16 end-to-end lands ~2e-3.  DRAM layouts are pre-permuted on the
host so every DMA run is one contiguous 5832B row per channel per
subtile (one packet per partition per transfer, ~26 B/ns/engine).  All
DMA is HWDGE: loads on the sync queue, stores on the scalar queue.

Engine assignment per subtile (9 d-rows x 36 patch columns):
  sync   : x load, att load (HWDGE)
  scalar : Sign(att) for the nonzero count; half the leaky-relus
           (Prelu); out store (HWDGE)
  tensor : 6 bf16 matmuls into 2 psum groups
  vector : segmented reduce -> 1/nz, t = (A+(b+1))+psum,
           pre = (psum+b)*t
  gpsimd : A = att*(1/nz) (also the voxel-order permutation);
           other half of the leaky-relus as (pre*0.2) max pre
"""

import os
import sys

import numpy as np

sys.path.insert(0, "/opt/trn_rl_repo")

# ---- geometry (hardcoded for this problem) ----
C = 128          # channels (in == out)
D = 36           # depth
HWFULL = 5184    # H*W = 72*72
PS = 9           # patch size
NDP = 4          # D // PS
NWP = 576        # HWFULL // PS  (patch columns)
NCORES = 8
IWG = NWP // NCORES   # 72 patch columns per core
HWL = IWG * PS        # 648 voxel columns per core
NSUB = 2              # split each iD block into halves along iW
IWT = IWG // NSUB     # 36 patch columns per subtile
FT = IWT * 81         # 2916 free elements per subtile
HWT = IWT * PS        # 324 voxel columns per subtile
MMN = 486             # matmul free dim: contiguous voxel slice
NMM = PS * HWT // MMN # 6 matmuls per subtile
NGRP = 2              # psum groups per subtile
MMG = NMM // NGRP     # 3 matmuls per psum group
BANK = 512            # fp32 elements per PSUM bank
NT = NDP * NSUB       # 8 subtiles per core

_NC_CACHE = {}
LAST_RESULT = None


def _build_nc(att_dtype="bfloat16", lrelu_engine="scalar", sign_engine="split_sv",
              amul_engine="gpsimd"):
    from contextlib import ExitStack

    import concourse.bacc as bacc
    import concourse.tile as tile
    from concourse import mybir

    f32 = mybir.dt.float32
    bf16 = mybir.dt.bfloat16
    AL = mybir.AluOpType
    AF = mybir.ActivationFunctionType
    adt = getattr(mybir.dt, att_dtype)

    nc = bacc.Bacc(
        "TRN2",
        target_bir_lowering=False,
        debug=False,
        enable_asserts=False,
        num_devices=NCORES,
    )
    x_d = nc.dram_tensor("x", [C, NT, PS * HWT], bf16, kind="ExternalInput").ap()
    a_d = nc.dram_tensor("att", [C, NT, FT], adt, kind="ExternalInput").ap()
    wt_d = nc.dram_tensor("wt", [C, C], bf16, kind="ExternalInput").ap()
    b_d = nc.dram_tensor("bias", [C, 2], f32, kind="ExternalInput").ap()
    o_d = nc.dram_tensor("out", [C, NT, PS * HWT], bf16, kind="ExternalOutput").ap()

    with tile.TileContext(nc) as tc, ExitStack() as ctx:
        const = ctx.enter_context(tc.tile_pool(name="const", bufs=1))
        wt_sb = const.tile([C, C], bf16)
        nc.sync.dma_start(wt_sb[:], wt_d[:])
        b_sb = const.tile([C, 2], f32)
        nc.sync.dma_start(b_sb[:], b_d[:])
        b_ap = b_sb[:, 0:1]
        bp1_ap = b_sb[:, 1:2]
        alpha_sb = const.tile([C, 1], f32)
        nc.vector.memset(alpha_sb[:], 0.2)
        zero_sb = const.tile([C, 1], f32)
        nc.vector.memset(zero_sb[:], 0.0)

        xp = ctx.enter_context(tc.tile_pool(name="xp", bufs=3))
        atp = ctx.enter_context(tc.tile_pool(name="atp", bufs=3))
        sgp = ctx.enter_context(tc.tile_pool(name="sgp", bufs=2))
        nzp = ctx.enter_context(tc.tile_pool(name="nzp", bufs=2))
        Apl = ctx.enter_context(tc.tile_pool(name="Apl", bufs=2))
        tpl = ctx.enter_context(tc.tile_pool(name="tpl", bufs=2))
        prp = ctx.enter_context(tc.tile_pool(name="prp", bufs=2))
        ovp = ctx.enter_context(tc.tile_pool(name="ovp", bufs=3))
        psp = ctx.enter_context(tc.tile_pool(name="psp", bufs=2, space="PSUM"))

        def issue_loads(sub):
            xt = xp.tile([C, PS * HWT], bf16, name=f"xt{sub}", tag="xt")
            nc.sync.dma_start(xt[:], x_d[:, sub, :])
            at = atp.tile([C, FT], adt, name=f"at{sub}", tag="at")
            nc.sync.dma_start(at[:], a_d[:, sub, :])
            return xt, at

        loaded = {0: issue_loads(0), 1: issue_loads(1)}

        for sub in range(NT):
            xt, at = loaded.pop(sub)
            if sub + 2 < NT:
                loaded[sub + 2] = issue_loads(sub + 2)

            # ---- nz = count_nonzero per patch:  sum |sign(att)| ----
            st = sgp.tile([C, FT], bf16)
            if sign_engine == "split_sg":
                seng = "scalar" if sub % 2 == 0 else "gpsimd"
            elif sign_engine == "split_sv":
                seng = "scalar" if sub % 2 == 0 else "vector"
            else:
                seng = sign_engine
            if seng == "scalar":
                nc.scalar.activation(st[:], at[:], AF.Sign)
            elif seng == "gpsimd":
                # Pool has no tensor_scalar; not_equal against a
                # stride-0 broadcast zero instead.
                zbc = zero_sb[:].broadcast_to((C, FT))
                nc.gpsimd.tensor_tensor(st[:], at[:], zbc, AL.not_equal)
            else:
                nc.vector.tensor_scalar(st[:], at[:], 0.0, None, AL.not_equal)
            nzv = nzp.tile([C, IWT], f32)
            nc.vector.tensor_reduce(
                nzv[:],
                st[:].rearrange("c (w k) -> c w k", k=81),
                mybir.AxisListType.X,
                AL.add,
                apply_absolute_value=True,
            )
            nzr = nzp.tile([C, IWT], f32)
            nc.vector.tensor_scalar_add(nzv[:], nzv[:], 1e-5)
            nc.vector.reciprocal_approx_fast(nzr[:], nzv[:])

            # ---- A = att * (1/nz), written in VOXEL order (p1, iW, p2) -
            # at free layout is (iW:36, p1:9, p2:9); read it strided in
            # (p1, iW, p2) order (innermost contiguous runs of 9), write
            # contiguous.  Everything downstream is then voxel-ordered.
            At = Apl.tile([C, FT], bf16)
            a3 = (
                at[:]
                .rearrange("c (iw p q) -> c iw p q", p=PS, q=PS)
                .transpose([0, 2, 1, 3])
            )  # dims (p1, iW, p2) steps (9, 81, 1)
            nzr3 = (
                nzr[:]
                .unsqueeze(1)
                .unsqueeze(3)
                .broadcast_to((C, PS, IWT, PS))
            )
            A3 = At[:].rearrange("c (p iw q) -> c p iw q", p=PS, q=PS)
            amul = nc.gpsimd if amul_engine == "gpsimd" else nc.vector
            amul.tensor_tensor(A3, a3, nzr3, AL.mult)

            # ---- GEMM: psum = W @ x, plain contiguous voxel slices ----
            pst = []
            for g in range(NGRP):
                ps_t = psp.tile([C, MMG * BANK], f32)  # 3 banks
                pst.append(ps_t)
                for m in range(MMG):
                    ch = g * MMG + m
                    nc.tensor.matmul(
                        ps_t[:, m * BANK : m * BANK + MMN],
                        wt_sb[:],
                        xt[:, ch * MMN : (ch + 1) * MMN],
                        start=True,
                        stop=True,
                    )

            # ---- t = (A + (b+1)) + psum ; pre = (psum + b) * t ----
            tt = tpl.tile([C, FT], f32)
            pre = prp.tile([C, FT], f32)
            for g in range(NGRP):
                ps_ap = (
                    pst[g][:]
                    .rearrange("c (m n) -> c m n", n=BANK)[:, :, 0:MMN]
                )  # [C, 3, 486]
                sl = slice(g * MMG * MMN, (g + 1) * MMG * MMN)
                A2 = At[:, sl].rearrange("c (m n) -> c m n", n=MMN)
                t2 = tt[:, sl].rearrange("c (m n) -> c m n", n=MMN)
                p2_ = pre[:, sl].rearrange("c (m n) -> c m n", n=MMN)
                nc.vector.scalar_tensor_tensor(
                    t2, A2, bp1_ap, ps_ap, AL.add, AL.add
                )
                nc.vector.scalar_tensor_tensor(
                    p2_, ps_ap, b_ap, t2, AL.add, AL.mult
                )

            # ---- out = lrelu(pre); voxel order ----
            ov = ovp.tile([C, PS * HWT], bf16)
            if lrelu_engine == "split_sv":
                leng = "scalar" if sub % 2 == 0 else "vector"
            else:
                leng = lrelu_engine
            if leng == "scalar":
                nc.scalar.activation(
                    ov[:], pre[:], AF.Prelu, alpha=alpha_sb[:, 0:1]
                )
            else:
                # lrelu(x) = (x*0.2) max x as one DVE scalar_tensor_tensor
                nc.vector.scalar_tensor_tensor(
                    ov[:], pre[:], 0.2, pre[:], AL.mult, AL.max
                )

            # ---- contiguous store on the scalar HWDGE queue ----
            nc.scalar.dma_start(o_d[:, sub, :], ov[:])

    nc.compile()
    return nc


def _get_nc(**kw):
    key = tuple(sorted(kw.items()))
    if key not in _NC_CACHE:
        _NC_CACHE[key] = _build_nc(**kw)
    return _NC_CACHE[key]


def kernel(x, attentions, W, b, **build_kw):
    global LAST_RESULT
    import ml_dtypes
    from concourse.bass_utils import run_bass_kernel_spmd

    bf16 = ml_dtypes.bfloat16
    att_np = {"bfloat16": bf16, "float8e4": ml_dtypes.float8_e4m3}[
        build_kw.get("att_dtype", "bfloat16")
    ]

    x = np.asarray(x, dtype=np.float32)
    attentions = np.asarray(attentions, dtype=np.float32)
    W = np.asarray(W, dtype=np.float32)
    b = np.asarray(b, dtype=np.float32)

    nc = _get_nc(**build_kw)

    # x: [C, NDP, PS, NCORES, NSUB, HWT] -> core-major, subtile-contiguous
    xs = (
        x.reshape(C, NDP, PS, NCORES, NSUB, HWT)
        .transpose(3, 0, 1, 4, 2, 5)
        .reshape(NCORES, C, NT, PS * HWT)
        .astype(bf16)
    )
    # att: [C, NDP, NCORES, NSUB, IWT*81] -> core-major, subtile-contiguous
    as_ = (
        attentions.reshape(C, NDP, NCORES, NSUB, FT)
        .transpose(2, 0, 1, 3, 4)
        .reshape(NCORES, C, NT, FT)
        .astype(att_np)
    )
    wt = np.ascontiguousarray(W.T).astype(bf16)
    bcol = np.ascontiguousarray(np.stack([b, b + 1.0], axis=1))

    in_maps = []
    for s in range(NCORES):
        in_maps.append(
            {
                "x": np.ascontiguousarray(xs[s]),
                "att": np.ascontiguousarray(as_[s]),
                "wt": wt,
                "bias": bcol,
            }
        )

    res = run_bass_kernel_spmd(
        nc,
        in_maps,
        core_ids=list(range(NCORES)),
        trace=bool(os.environ.get("BASS_TRACE")),
    )
    LAST_RESULT = res

    out = np.empty((NCORES, C, NT, PS * HWT), dtype=np.float32)
    for s in range(NCORES):
        out[s] = res.results[s]["out"].astype(np.float32)
    # inverse of the x permutation
    full = (
        out.reshape(NCORES, C, NDP, NSUB, PS, HWT)
        .transpose(1, 2, 4, 0, 3, 5)
        .reshape(1, C, D, HWFULL)
    )
    return full
